# revision 23
# baseline (speedup 1.0000x reference)
"""GAT-style masked self-attention (B=4, N=4096, D=128) on 8 trn2 NeuronCores.

reference:
    scores = X @ X^T / sqrt(D)            [B, N, N]
    masked = where(adj > 0, scores, -1e12)
    attn   = softmax(masked, axis=2)
    out    = attn @ X                     [B, N, D]

Sharding: 8 cores <- (batch b, row-half h); each core handles 2048 rows
of one batch element against all 4096 keys. No collectives: every core
produces its own 2048x128 output slice.

Device algorithm (per core), orientation "S^T" (keys on partitions):
  - score matmul (float32r, full PE rate): psS = XT[:,k128].T @ XTrows[:,blk]
  - ACT evicts PSUM with exp fused: p = exp(scale*psS - 8)  (fp16; the -8
    keeps probs inside fp16 range and cancels in the softmax ratio)
  - DVE applies the 0/1 mask in one big 2x-mode multiply per 8-key-tile
    super group: ptm = p * adjT
  - AV matmul with the denominator fused via an appended ones-column:
      psO[rc] (+)= ptm[:, k, rc128].T @ [X_k | 1]   accumulated over k
      out = psO[:, :128] * (1 / psO[:, 128])        row-wise normalize
  - softmax shift-invariance makes a row-max pass unnecessary:
    scores*scale are bounded (~|s|<16), exp stays well inside fp32 range.
  - row blocks are software-pipelined: block i runs scores/exp/mask while
    block i-1 runs its AV matmuls (ptm double-buffered); AV matmuls are
    emitted first within each group so PE covers the ACT drain. The last
    two blocks are 256 rows so the final (unoverlapped) AV drain is short.
"""

import math
import sys

sys.path.insert(0, "/opt/trn_rl_repo")

import numpy as np

B, N, D = 4, 4096, 128
R = N // 2            # rows per core
NK = N // 128         # 32 key tiles
RB = 512              # row granularity of the host-packed mask layout
NRB = R // RB
SG = 8                # key tiles per super group (one mask DMA / mask mul)
NSG = NK // SG
SCALE = 1.0 / math.sqrt(D)
EXP_BIAS = -8.0       # exp(s*scale - 8): keeps probs in fp16 range; cancels

# row blocks (offset, size): last two halved to shorten the AV drain tail
BLOCKS = [(0, 512), (512, 512), (1024, 512), (1536, 256), (1792, 256)]

CFG = dict(
    score_dt="float16",
    p_dt="float16",
    adj_dt="float16",
    ptm_bufs=2,
    kg=2,                 # key tiles per PSUM score tile (= ACT evict batch)
    psum_s_bufs=2,
    adj_bufs=4,
)

_CACHE = {}


def _build_nc(cfg):
    from concourse import bacc
    import concourse.mybir as mybir
    from concourse.tile import TileContext

    dt = mybir.dt
    score_dt = getattr(dt, cfg["score_dt"])
    p_dt = getattr(dt, cfg["p_dt"])
    adj_dt = getattr(dt, cfg["adj_dt"])
    kg = cfg["kg"]

    nc = bacc.Bacc(None, target_bir_lowering=False)

    xt_d = nc.dram_tensor("xt", [D, N], score_dt, kind="ExternalInput")
    xtr_d = nc.dram_tensor("xtr", [D, R], score_dt, kind="ExternalInput")
    xaug_d = nc.dram_tensor("xaug", [N, D + 1], p_dt, kind="ExternalInput")
    # 0/1 mask, host-packed as [rb, key_in_tile, key_tile, row_in_block]
    adj_d = nc.dram_tensor("adjt", [NRB, 128, NK, RB], adj_dt, kind="ExternalInput")
    o_d = nc.dram_tensor("o", [R, D], dt.float32, kind="ExternalOutput")

    def adj_src(off, bs, sg0, nsg):
        rb0, r0 = off // RB, off % RB
        return adj_d[rb0, :, sg0 * SG:(sg0 + nsg) * SG, r0:r0 + bs]

    with TileContext(nc) as tc:
        with (
            tc.tile_pool(name="singles", bufs=1) as singles,
            tc.tile_pool(name="ptm", bufs=cfg["ptm_bufs"]) as ptm_pool,
            tc.tile_pool(name="adj", bufs=cfg["adj_bufs"]) as adj_pool,
            tc.tile_pool(name="pe", bufs=2) as pe_pool,
            tc.tile_pool(name="outs", bufs=4) as out_pool,
            tc.tile_pool(name="small", bufs=4) as small_pool,
            tc.tile_pool(name="psS", bufs=cfg["psum_s_bufs"], space="PSUM") as psS_pool,
            tc.tile_pool(name="psO", bufs=4, space="PSUM") as psO_pool,
        ):
            ebias = singles.tile([128, 1], mybir.dt.float32)
            nc.vector.memset(ebias[:], EXP_BIAS)
            # warm the exp table while the init DMAs stream in
            warm = small_pool.tile([128, 1], mybir.dt.float32, tag="warm")
            nc.vector.memset(warm[:], 0.0)
            warm2 = small_pool.tile([128, 1], mybir.dt.float32, tag="warm")
            nc.scalar.activation(
                warm2[:], warm[:], mybir.ActivationFunctionType.Exp, scale=1.0
            )

            # init DMAs staggered by first consumption. sync HWDGE ring:
            # xt/xtr pieces the first score matmuls need, then the rest of
            # xt. SWDGE ring (independent): first mask chunk, rest of xtr,
            # xaug.
            xt_sb = singles.tile([D, N], score_dt)
            xtr_sb = singles.tile([D, R], score_dt)
            nc.sync.dma_start(out=xtr_sb[:, 0:512], in_=xtr_d[:, 0:512])
            nc.sync.dma_start(out=xt_sb[:, 0:1024], in_=xt_d[:, 0:1024])
            nc.sync.dma_start(out=xt_sb[:, 1024:2048], in_=xt_d[:, 1024:2048])
            adj0 = adj_pool.tile([128, SG, RB], adj_dt, tag="adj", name="adj_0_0")
            nc.sync.dma_start(out=adj0[:, :, 0:BLOCKS[0][1]],
                              in_=adj_src(0, BLOCKS[0][1], 0, 1))
            nc.sync.dma_start(out=xt_sb[:, 2048:4096], in_=xt_d[:, 2048:4096])
            xaug_sb = singles.tile([128, NK, D + 1], p_dt)
            nc.gpsimd.dma_start(
                out=xaug_sb[:],
                in_=xaug_d[:, :].rearrange("(t p) d -> p t d", p=128),
            )
            nc.gpsimd.dma_start(out=xtr_sb[:, 512:2048], in_=xtr_d[:, 512:2048])

            NB = len(BLOCKS)
            ptm_prev = None
            bs_prev = None
            off_prev = None
            for phase in range(NB + 1):
                ptm_cur = None
                psO = None
                adj_sbs = []
                if phase < NB:
                    off, bs = BLOCKS[phase]
                    ptm_cur = ptm_pool.tile([128, NK, bs], p_dt, tag="ptm",
                                            name=f"ptm_{phase}")
                    if phase == 0:
                        # 1MB chunks: the first block's mask demand starts
                        # before the DMA stream has caught up
                        adj_sbs.append((adj0, 0))
                        for sg in range(1, NSG):
                            a = adj_pool.tile([128, SG, bs], adj_dt, tag="adj",
                                              name=f"adj_{phase}_{sg}")
                            nc.sync.dma_start(out=a[:],
                                              in_=adj_src(off, bs, sg, 1))
                            adj_sbs.append((a, 0))
                    else:
                        # 2MB chunks for DMA efficiency
                        per = 2 if bs == 512 else 4   # super groups per chunk
                        for c in range(NSG // per):
                            a = adj_pool.tile([128, per * SG, bs], adj_dt,
                                              tag="adj", name=f"adj_{phase}_{c}")
                            nc.sync.dma_start(out=a[:],
                                              in_=adj_src(off, bs, c * per, per))
                            for i in range(per):
                                adj_sbs.append((a, i))
                if phase >= 1:
                    psO = [
                        psO_pool.tile(
                            [128, D + 1], mybir.dt.float32,
                            tag="psO", name=f"psO_{phase}_{rc}",
                        )
                        for rc in range(bs_prev // 128)
                    ]

                if phase == NB:
                    # drain: rc-major AV bursts so each psO finishes early
                    # and its normalize/store overlaps the next burst
                    for rc in range(bs_prev // 128):
                        for k in range(NK):
                            nc.tensor.matmul(
                                psO[rc][:, :],
                                lhsT=ptm_prev[:, k, rc * 128:(rc + 1) * 128],
                                rhs=xaug_sb[:, k, :],
                                start=(k == 0),
                                stop=(k == NK - 1),
                            )
                        recip = small_pool.tile([128, 1], mybir.dt.float32,
                                                tag="recip", name=f"recipd_{rc}")
                        nc.vector.reciprocal(recip[:], psO[rc][:, D:D + 1])
                        o_sb = out_pool.tile([128, D], mybir.dt.float32, tag="o",
                                             name=f"od_{rc}")
                        nc.vector.tensor_scalar_mul(o_sb[:], psO[rc][:, 0:D],
                                                    recip[:])
                        r0 = off_prev + rc * 128
                        nc.sync.dma_start(out=o_d[r0:r0 + 128, :], in_=o_sb[:])
                    break

                kg_b = kg * (RB // bs)   # keep kg_b*bs = 1024 elems per evict
                for sg in range(NSG):
                    pet = pe_pool.tile([128, SG, bs], p_dt, tag="pe",
                                       name=f"pe_{phase}_{sg}")
                    for kgi in range(SG // kg_b):
                        # AV matmuls for the previous block first: PE has
                        # work while ACT drains this group's scores.
                        if phase >= 1:
                            for j in range(kg_b):
                                k = sg * SG + kgi * kg_b + j
                                for rc in range(bs_prev // 128):
                                    nc.tensor.matmul(
                                        psO[rc][:, :],
                                        lhsT=ptm_prev[:, k, rc * 128:(rc + 1) * 128],
                                        rhs=xaug_sb[:, k, :],
                                        start=(k == 0),
                                        stop=(k == NK - 1),
                                    )
                        ps = psS_pool.tile([128, kg_b, bs], mybir.dt.float32,
                                           tag="psS", name=f"psS_{phase}_{sg}_{kgi}")
                        for j in range(kg_b):
                            k = sg * SG + kgi * kg_b + j
                            nc.tensor.matmul(
                                ps[:, j, :],
                                lhsT=xt_sb[:, k * 128:(k + 1) * 128],
                                rhs=xtr_sb[:, off:off + bs],
                                start=True,
                                stop=True,
                            )
                        # evict PSUM with exp fused; mask comes after
                        nc.scalar.activation(
                            pet[:, kgi * kg_b:(kgi + 1) * kg_b, :],
                            ps[:, :, :],
                            mybir.ActivationFunctionType.Exp,
                            bias=ebias[:],
                            scale=SCALE,
                        )
                    k0 = sg * SG
                    a, si = adj_sbs[sg]
                    nc.vector.tensor_mul(
                        ptm_cur[:, k0:k0 + SG, :],
                        pet[:, :, :],
                        a[:, si * SG:(si + 1) * SG, 0:bs],
                    )
                if phase >= 1:
                    for rc in range(bs_prev // 128):
                        recip = small_pool.tile([128, 1], mybir.dt.float32,
                                                tag="recip",
                                                name=f"recip_{phase}_{rc}")
                        nc.vector.reciprocal(recip[:], psO[rc][:, D:D + 1])
                        o_sb = out_pool.tile([128, D], mybir.dt.float32, tag="o",
                                             name=f"o_{phase}_{rc}")
                        nc.vector.tensor_scalar_mul(o_sb[:], psO[rc][:, 0:D],
                                                    recip[:])
                        r0 = off_prev + rc * 128
                        nc.sync.dma_start(out=o_d[r0:r0 + 128, :], in_=o_sb[:])
                ptm_prev = ptm_cur
                bs_prev = bs
                off_prev = off
    nc.finalize()
    return nc


def _get_nc():
    key = tuple(sorted(CFG.items()))
    if key not in _CACHE:
        _CACHE[key] = _build_nc(CFG)
    return _CACHE[key]


def _np_dt(name):
    import ml_dtypes

    return {
        "float32": np.float32,
        "float32r": np.float32,
        "bfloat16": ml_dtypes.bfloat16,
        "float16": np.float16,
    }[name]


def make_in_maps(input, adj):
    """Host-side shard/layout prep: one input map per core."""
    input = np.asarray(input, dtype=np.float32)
    adj = np.asarray(adj)
    score_np = _np_dt(CFG["score_dt"])
    p_np = _np_dt(CFG["p_dt"])
    adj_np = _np_dt(CFG["adj_dt"])

    in_maps = []
    for core in range(8):
        b, h = core // 2, core % 2
        xb = input[b]                                    # [N, D]
        xt = np.ascontiguousarray(xb.T).astype(score_np, copy=False)
        xtr = np.ascontiguousarray(xb.T[:, h * R:(h + 1) * R]).astype(
            score_np, copy=False
        )
        xaug = np.concatenate([xb, np.ones((N, 1), np.float32)], axis=1)
        xaug = np.ascontiguousarray(xaug).astype(p_np)
        s = adj[b][h * R:(h + 1) * R, :]                 # [R rows, N cols]
        # multiplicative 0/1 mask; adjt[rb, p, k, r] = (s[rb*512+r, k*128+p]>0)
        adjt = np.ascontiguousarray(
            (s > 0).astype(adj_np).reshape(NRB, RB, NK, 128).transpose(0, 3, 2, 1)
        )
        in_maps.append({"xt": xt, "xtr": xtr, "xaug": xaug, "adjt": adjt})
    return in_maps


def run_device(in_maps, trace=False, trace_cores=None):
    import concourse.bass_utils as bass_utils

    if trace:
        bass_utils.upload_artifacts = lambda tmpdir: ""  # no bucket in sandbox
    nc = _get_nc()
    return bass_utils.run_bass_kernel_spmd(
        nc, in_maps, list(range(8)), trace=trace, trace_cores=trace_cores
    )


def kernel(input, adj):
    res = run_device(make_in_maps(input, adj))
    out = np.empty((B, N, D), dtype=np.float32)
    for core in range(8):
        b, h = core // 2, core % 2
        out[b, h * R:(h + 1) * R, :] = res.results[core]["o"]
    return out


# revision 24
# speedup vs baseline: 1.0304x; 1.0304x over previous
"""GAT-style masked self-attention (B=4, N=4096, D=128) on 8 trn2 NeuronCores.

reference:
    scores = X @ X^T / sqrt(D)            [B, N, N]
    masked = where(adj > 0, scores, -1e12)
    attn   = softmax(masked, axis=2)
    out    = attn @ X                     [B, N, D]

Sharding: 8 cores <- (batch b, row-half h); each core handles 2048 rows
of one batch element against all 4096 keys. No collectives: every core
produces its own 2048x128 output slice.

Device algorithm (per core), orientation "S^T" (keys on partitions):
  - score matmul (float32r, full PE rate): psS = XT[:,k128].T @ XTrows[:,blk]
  - ACT evicts PSUM with exp fused: p = exp(scale*psS - 8)  (fp16; the -8
    keeps probs inside fp16 range and cancels in the softmax ratio)
  - DVE applies the 0/1 mask in one big 2x-mode multiply per 8-key-tile
    super group: ptm = p * adjT
  - AV matmul with the denominator fused via an appended ones-column:
      psO[rc] (+)= ptm[:, k, rc128].T @ [X_k | 1]   accumulated over k
      out = psO[:, :128] * (1 / psO[:, 128])        row-wise normalize
  - softmax shift-invariance makes a row-max pass unnecessary:
    scores*scale are bounded (~|s|<16), exp stays well inside fp32 range.
  - row blocks are software-pipelined: block i runs scores/exp/mask while
    block i-1 runs its AV matmuls (ptm double-buffered); AV matmuls are
    emitted first within each group so PE covers the ACT drain. The last
    two blocks are 256 rows so the final (unoverlapped) AV drain is short.
"""

import math
import sys

sys.path.insert(0, "/opt/trn_rl_repo")

import numpy as np

B, N, D = 4, 4096, 128
R = N // 2            # rows per core
NK = N // 128         # 32 key tiles
RB = 512              # row granularity of the host-packed mask layout
NRB = R // RB
SG = 8                # key tiles per super group (one mask DMA / mask mul)
NSG = NK // SG
SCALE = 1.0 / math.sqrt(D)
EXP_BIAS = -8.0       # exp(s*scale - 8): keeps probs in fp16 range; cancels

# row blocks (offset, size): last two halved to shorten the AV drain tail
BLOCKS = [(0, 512), (512, 512), (1024, 512), (1536, 256), (1792, 256)]

CFG = dict(
    score_dt="float16",
    p_dt="float16",
    adj_dt="float16",
    ptm_bufs=2,
    kg=2,                 # key tiles per PSUM score tile (= ACT evict batch)
    psum_s_bufs=2,
    adj_bufs=4,
)

_CACHE = {}


def _build_nc(cfg):
    from concourse import bacc
    import concourse.mybir as mybir
    from concourse.tile import TileContext

    dt = mybir.dt
    score_dt = getattr(dt, cfg["score_dt"])
    p_dt = getattr(dt, cfg["p_dt"])
    adj_dt = getattr(dt, cfg["adj_dt"])
    kg = cfg["kg"]

    nc = bacc.Bacc(None, target_bir_lowering=False)

    xt_d = nc.dram_tensor("xt", [D, N], score_dt, kind="ExternalInput")
    xtr_d = nc.dram_tensor("xtr", [D, R], score_dt, kind="ExternalInput")
    xaug_d = nc.dram_tensor("xaug", [N, D + 1], p_dt, kind="ExternalInput")
    # 0/1 mask, host-packed as [rb, key_in_tile, key_tile, row_in_block]
    adj_d = nc.dram_tensor("adjt", [NRB, 128, NK, RB], adj_dt, kind="ExternalInput")
    o_d = nc.dram_tensor("o", [R, D], dt.float32, kind="ExternalOutput")

    def adj_src(off, bs, sg0, nsg):
        rb0, r0 = off // RB, off % RB
        return adj_d[rb0, :, sg0 * SG:(sg0 + nsg) * SG, r0:r0 + bs]

    with TileContext(nc) as tc:
        with (
            tc.tile_pool(name="singles", bufs=1) as singles,
            tc.tile_pool(name="ptm", bufs=cfg["ptm_bufs"]) as ptm_pool,
            tc.tile_pool(name="adj", bufs=cfg["adj_bufs"]) as adj_pool,
            tc.tile_pool(name="pe", bufs=2) as pe_pool,
            tc.tile_pool(name="outs", bufs=4) as out_pool,
            tc.tile_pool(name="small", bufs=4) as small_pool,
            tc.tile_pool(name="psS", bufs=cfg["psum_s_bufs"], space="PSUM") as psS_pool,
            tc.tile_pool(name="psO", bufs=4, space="PSUM") as psO_pool,
        ):
            ebias = singles.tile([128, 1], mybir.dt.float32)
            nc.vector.memset(ebias[:], EXP_BIAS)
            # warm the exp table while the init DMAs stream in
            warm = small_pool.tile([128, 1], mybir.dt.float32, tag="warm")
            nc.vector.memset(warm[:], 0.0)
            warm2 = small_pool.tile([128, 1], mybir.dt.float32, tag="warm")
            nc.scalar.activation(
                warm2[:], warm[:], mybir.ActivationFunctionType.Exp, scale=1.0
            )

            # init DMAs staggered by first consumption. sync HWDGE ring:
            # xt/xtr pieces the first score matmuls need, then the rest of
            # xt. SWDGE ring (independent): first mask chunk, rest of xtr,
            # xaug.
            xt_sb = singles.tile([D, N], score_dt)
            xtr_sb = singles.tile([D, R], score_dt)
            nc.sync.dma_start(out=xtr_sb[:, 0:512], in_=xtr_d[:, 0:512])
            nc.sync.dma_start(out=xt_sb[:, 0:1024], in_=xt_d[:, 0:1024])
            nc.sync.dma_start(out=xt_sb[:, 1024:2048], in_=xt_d[:, 1024:2048])
            adj0 = adj_pool.tile([128, SG, RB], adj_dt, tag="adj", name="adj_0_0")
            nc.sync.dma_start(out=adj0[:, :, 0:BLOCKS[0][1]],
                              in_=adj_src(0, BLOCKS[0][1], 0, 1))
            nc.sync.dma_start(out=xt_sb[:, 2048:4096], in_=xt_d[:, 2048:4096])
            xaug_sb = singles.tile([128, NK, D + 1], p_dt)
            nc.gpsimd.dma_start(
                out=xaug_sb[:],
                in_=xaug_d[:, :].rearrange("(t p) d -> p t d", p=128),
            )
            nc.gpsimd.dma_start(out=xtr_sb[:, 512:2048], in_=xtr_d[:, 512:2048])

            NB = len(BLOCKS)
            ptm_prev = None
            bs_prev = None
            off_prev = None
            for phase in range(NB + 1):
                ptm_cur = None
                psO = None
                adj_sbs = []
                if phase < NB:
                    off, bs = BLOCKS[phase]
                    ptm_cur = ptm_pool.tile([128, NK, bs], p_dt, tag="ptm",
                                            name=f"ptm_{phase}")
                    if phase == 0:
                        # 1MB chunks: the first block's mask demand starts
                        # before the DMA stream has caught up
                        adj_sbs.append((adj0, 0))
                        for sg in range(1, NSG):
                            a = adj_pool.tile([128, SG, bs], adj_dt, tag="adj",
                                              name=f"adj_{phase}_{sg}")
                            nc.sync.dma_start(out=a[:],
                                              in_=adj_src(off, bs, sg, 1))
                            adj_sbs.append((a, 0))
                    else:
                        # 2MB chunks for DMA efficiency
                        per = 1 if bs == 512 else 2   # ~1MB per chunk
                        for c in range(NSG // per):
                            a = adj_pool.tile([128, per * SG, bs], adj_dt,
                                              tag="adj", name=f"adj_{phase}_{c}")
                            nc.sync.dma_start(out=a[:],
                                              in_=adj_src(off, bs, c * per, per))
                            for i in range(per):
                                adj_sbs.append((a, i))
                if phase >= 1:
                    psO = [
                        psO_pool.tile(
                            [128, D + 1], mybir.dt.float32,
                            tag="psO", name=f"psO_{phase}_{rc}",
                        )
                        for rc in range(bs_prev // 128)
                    ]

                if phase == NB:
                    # drain: rc-major AV bursts so each psO finishes early
                    # and its normalize/store overlaps the next burst
                    for rc in range(bs_prev // 128):
                        for k in range(NK):
                            nc.tensor.matmul(
                                psO[rc][:, :],
                                lhsT=ptm_prev[:, k, rc * 128:(rc + 1) * 128],
                                rhs=xaug_sb[:, k, :],
                                start=(k == 0),
                                stop=(k == NK - 1),
                            )
                        recip = small_pool.tile([128, 1], mybir.dt.float32,
                                                tag="recip", name=f"recipd_{rc}")
                        nc.vector.reciprocal(recip[:], psO[rc][:, D:D + 1])
                        o_sb = out_pool.tile([128, D], mybir.dt.float32, tag="o",
                                             name=f"od_{rc}")
                        nc.vector.tensor_scalar_mul(o_sb[:], psO[rc][:, 0:D],
                                                    recip[:])
                        r0 = off_prev + rc * 128
                        nc.sync.dma_start(out=o_d[r0:r0 + 128, :], in_=o_sb[:])
                    break

                kg_b = kg * (RB // bs)   # keep kg_b*bs = 1024 elems per evict
                for sg in range(NSG):
                    pet = pe_pool.tile([128, SG, bs], p_dt, tag="pe",
                                       name=f"pe_{phase}_{sg}")
                    for kgi in range(SG // kg_b):
                        # AV matmuls for the previous block first: PE has
                        # work while ACT drains this group's scores.
                        if phase >= 1:
                            for j in range(kg_b):
                                k = sg * SG + kgi * kg_b + j
                                for rc in range(bs_prev // 128):
                                    nc.tensor.matmul(
                                        psO[rc][:, :],
                                        lhsT=ptm_prev[:, k, rc * 128:(rc + 1) * 128],
                                        rhs=xaug_sb[:, k, :],
                                        start=(k == 0),
                                        stop=(k == NK - 1),
                                    )
                        ps = psS_pool.tile([128, kg_b, bs], mybir.dt.float32,
                                           tag="psS", name=f"psS_{phase}_{sg}_{kgi}")
                        for j in range(kg_b):
                            k = sg * SG + kgi * kg_b + j
                            nc.tensor.matmul(
                                ps[:, j, :],
                                lhsT=xt_sb[:, k * 128:(k + 1) * 128],
                                rhs=xtr_sb[:, off:off + bs],
                                start=True,
                                stop=True,
                            )
                        # evict PSUM with exp fused; mask comes after
                        nc.scalar.activation(
                            pet[:, kgi * kg_b:(kgi + 1) * kg_b, :],
                            ps[:, :, :],
                            mybir.ActivationFunctionType.Exp,
                            bias=ebias[:],
                            scale=SCALE,
                        )
                    k0 = sg * SG
                    a, si = adj_sbs[sg]
                    nc.vector.tensor_mul(
                        ptm_cur[:, k0:k0 + SG, :],
                        pet[:, :, :],
                        a[:, si * SG:(si + 1) * SG, 0:bs],
                    )
                if phase >= 1:
                    for rc in range(bs_prev // 128):
                        recip = small_pool.tile([128, 1], mybir.dt.float32,
                                                tag="recip",
                                                name=f"recip_{phase}_{rc}")
                        nc.vector.reciprocal(recip[:], psO[rc][:, D:D + 1])
                        o_sb = out_pool.tile([128, D], mybir.dt.float32, tag="o",
                                             name=f"o_{phase}_{rc}")
                        nc.vector.tensor_scalar_mul(o_sb[:], psO[rc][:, 0:D],
                                                    recip[:])
                        r0 = off_prev + rc * 128
                        nc.sync.dma_start(out=o_d[r0:r0 + 128, :], in_=o_sb[:])
                ptm_prev = ptm_cur
                bs_prev = bs
                off_prev = off
    nc.finalize()
    return nc


def _get_nc():
    key = tuple(sorted(CFG.items()))
    if key not in _CACHE:
        _CACHE[key] = _build_nc(CFG)
    return _CACHE[key]


def _np_dt(name):
    import ml_dtypes

    return {
        "float32": np.float32,
        "float32r": np.float32,
        "bfloat16": ml_dtypes.bfloat16,
        "float16": np.float16,
    }[name]


def make_in_maps(input, adj):
    """Host-side shard/layout prep: one input map per core."""
    input = np.asarray(input, dtype=np.float32)
    adj = np.asarray(adj)
    score_np = _np_dt(CFG["score_dt"])
    p_np = _np_dt(CFG["p_dt"])
    adj_np = _np_dt(CFG["adj_dt"])

    in_maps = []
    for core in range(8):
        b, h = core // 2, core % 2
        xb = input[b]                                    # [N, D]
        xt = np.ascontiguousarray(xb.T).astype(score_np, copy=False)
        xtr = np.ascontiguousarray(xb.T[:, h * R:(h + 1) * R]).astype(
            score_np, copy=False
        )
        xaug = np.concatenate([xb, np.ones((N, 1), np.float32)], axis=1)
        xaug = np.ascontiguousarray(xaug).astype(p_np)
        s = adj[b][h * R:(h + 1) * R, :]                 # [R rows, N cols]
        # multiplicative 0/1 mask; adjt[rb, p, k, r] = (s[rb*512+r, k*128+p]>0)
        adjt = np.ascontiguousarray(
            (s > 0).astype(adj_np).reshape(NRB, RB, NK, 128).transpose(0, 3, 2, 1)
        )
        in_maps.append({"xt": xt, "xtr": xtr, "xaug": xaug, "adjt": adjt})
    return in_maps


def run_device(in_maps, trace=False, trace_cores=None):
    import concourse.bass_utils as bass_utils

    if trace:
        bass_utils.upload_artifacts = lambda tmpdir: ""  # no bucket in sandbox
    nc = _get_nc()
    return bass_utils.run_bass_kernel_spmd(
        nc, in_maps, list(range(8)), trace=trace, trace_cores=trace_cores
    )


def kernel(input, adj):
    res = run_device(make_in_maps(input, adj))
    out = np.empty((B, N, D), dtype=np.float32)
    for core in range(8):
        b, h = core // 2, core % 2
        out[b, h * R:(h + 1) * R, :] = res.results[core]["o"]
    return out
